# revision 1
# baseline (speedup 1.0000x reference)
"""Trainium2 Bass kernel for 4D cubic B-spline grid evaluation.

Problem: for each of 65536 query coords u in [0,1)^4, evaluate a uniform cubic
B-spline over an (8,16,16,16) control grid with 32 channels and linear-
extrapolation padding -> output (65536, 32) f32.

Strategy (data-parallel over the query batch, 8 cores x 8192 queries):
  * The linear-extrapolation grid padding is folded into transformed boundary
    weights, so no padded grid is ever materialized.
  * The grid is re-laid once in DRAM as a w-unfolded table: unit (t,d,h,wc) =
    the 4-wide w-window starting at clamped cell wc, i.e. 4x32ch = 512B
    contiguous.  26624 units, so indices fit dma_gather's int16.
  * Per query the other three dims contribute 4x4x4 = 64 units, gathered with
    SWDGE dma_gather (512B descriptors run at full DMA-bus rate; 8 sub-gathers
    of 1024 indices per 128-query tile since the descriptor ring holds 1024).
  * Separable weighted reduction (w, then h, d, t): the Scalar engine (ACT)
    computes the per-partition-scalar products of the two big stages (it owns
    the strided reads), the Vector engine (DVE) does contiguous adds plus the
    small-stage FMA chains, so the whole reduction hides under the gather DMA.
"""

import numpy as np

import concourse.bacc as bacc
import concourse.bass as bass
import concourse.mybir as mybir
import concourse.tile as tile
from concourse.bass_utils import run_bass_kernel_spmd

P = 128              # partitions / queries per tile
NT = 64              # tiles per core
BSH = P * NT         # 8192 queries per core
NCORES = 8
B = BSH * NCORES     # 65536
C = 32
SIZES = (8, 16, 16, 16)          # t, d, h, w control-point resolution
WCELLS = 13                      # distinct clamped w-window starts (0..12)
UNIT = 4 * 32                    # one gather unit: 4 w-points x 32 ch
NUNITS = 8 * 16 * 16 * WCELLS    # 26624 (< 32767, fits int16 indices)
F32 = mybir.dt.float32
I32 = mybir.dt.int32
I16 = mybir.dt.int16

_CACHED_NC = None


def _cubic_weights(nc, pool, f, nt):
    """Emit DVE ops computing the 4 cubic B-spline weights of fractional
    position tile `f` ([P, nt] f32).  Returns 4 tiles [P, nt]."""
    v = nc.vector
    A = mybir.AluOpType
    f2 = pool.tile([P, nt], F32, tag="f2")
    f3 = pool.tile([P, nt], F32, tag="f3")
    v.tensor_tensor(out=f2[:], in0=f[:], in1=f[:], op=A.mult)
    v.tensor_tensor(out=f3[:], in0=f2[:], in1=f[:], op=A.mult)
    w0 = pool.tile([P, nt], F32, tag="w0")
    w1 = pool.tile([P, nt], F32, tag="w1")
    w2 = pool.tile([P, nt], F32, tag="w2")
    w3 = pool.tile([P, nt], F32, tag="w3")
    tmp = pool.tile([P, nt], F32, tag="wtmp")
    # w0 = (1-f)^3/6 = -(f-1)^3/6
    v.tensor_scalar(out=tmp[:], in0=f[:], scalar1=1.0, scalar2=None, op0=A.subtract)
    v.tensor_tensor(out=w0[:], in0=tmp[:], in1=tmp[:], op=A.mult)
    v.tensor_tensor(out=w0[:], in0=w0[:], in1=tmp[:], op=A.mult)
    v.tensor_scalar(out=w0[:], in0=w0[:], scalar1=-1.0 / 6.0, scalar2=None, op0=A.mult)
    # w1 = 2/3 - f2 + f3/2  ->  (f3*0.5 - f2) + 2/3
    v.scalar_tensor_tensor(out=w1[:], in0=f3[:], scalar=0.5, in1=f2[:],
                           op0=A.mult, op1=A.subtract)
    v.tensor_scalar(out=w1[:], in0=w1[:], scalar1=2.0 / 3.0, scalar2=None, op0=A.add)
    # w2 = 1/6 + (f + f2 - f3)/2
    v.tensor_tensor(out=w2[:], in0=f[:], in1=f2[:], op=A.add)
    v.tensor_tensor(out=w2[:], in0=w2[:], in1=f3[:], op=A.subtract)
    v.tensor_scalar(out=w2[:], in0=w2[:], scalar1=0.5, scalar2=1.0 / 6.0,
                    op0=A.mult, op1=A.add)
    # w3 = f3/6
    v.tensor_scalar(out=w3[:], in0=f3[:], scalar1=1.0 / 6.0, scalar2=None, op0=A.mult)
    return w0, w1, w2, w3


def _build_nc(no_reduce=False, no_gather=False):
    nc = bacc.Bacc("TRN2", target_bir_lowering=False, debug=False,
                   num_devices=NCORES)
    u_in = nc.dram_tensor("u", [BSH, 4], F32, kind="ExternalInput")
    g_in = nc.dram_tensor("grid", [SIZES[0] * SIZES[1] * SIZES[2] * SIZES[3], C],
                          F32, kind="ExternalInput")
    out = nc.dram_tensor("out", [BSH, C], F32, kind="ExternalOutput")
    utab = nc.dram_tensor("utab", [NUNITS, UNIT], F32)
    bscr = nc.dram_tensor("bscr", [BSH], I16)

    v = nc.vector
    A = mybir.AluOpType

    with tile.TileContext(nc) as tc:
        with (
            tc.tile_pool(name="persist", bufs=1) as pp,
            tc.tile_pool(name="scratch", bufs=2) as sp,
            tc.tile_pool(name="gather", bufs=3) as gp,
            tc.tile_pool(name="red", bufs=2) as rp,
            tc.tile_pool(name="prod", bufs=3) as prp,
        ):
            # ---------------- Phase A: per-query prep for the whole shard ---
            U = pp.tile([P, NT, 4], F32)
            # query q = t*128 + p  ->  partition p, slot t
            nc.sync.dma_start(
                out=U[:], in_=u_in[:].rearrange("(t p) d -> p t d", p=P))

            # per-dim transformed weights [P, NT, 4] and window starts [P, NT]
            Wd_tiles = []
            O_tiles = []
            for dim in range(4):
                n = float(SIZES[dim])
                s = sp.tile([P, NT], F32, tag="s")
                v.tensor_scalar(out=s[:], in0=U[:, :, dim], scalar1=n - 1.0,
                                scalar2=None, op0=A.mult)
                # floor(s) via int cast round-trip; i = r - (s < r) is correct
                # whether the f32->i32 cast truncates or rounds-to-nearest
                # (s >= 0 always here).
                ri = sp.tile([P, NT], I32, tag="ri")
                v.tensor_copy(out=ri[:], in_=s[:])
                rf = sp.tile([P, NT], F32, tag="rf")
                v.tensor_copy(out=rf[:], in_=ri[:])
                flt = sp.tile([P, NT], F32, tag="flt")
                v.tensor_tensor(out=flt[:], in0=s[:], in1=rf[:], op=A.is_lt)
                ifl = sp.tile([P, NT], F32, tag="ifl")
                v.tensor_tensor(out=ifl[:], in0=rf[:], in1=flt[:], op=A.subtract)
                ic = sp.tile([P, NT], F32, tag="ic")
                v.tensor_scalar(out=ic[:], in0=ifl[:], scalar1=n - 2.0,
                                scalar2=None, op0=A.min)
                f = sp.tile([P, NT], F32, tag="f")
                v.tensor_tensor(out=f[:], in0=s[:], in1=ic[:], op=A.subtract)
                mL = sp.tile([P, NT], F32, tag="mL")
                v.tensor_scalar(out=mL[:], in0=ic[:], scalar1=0.0, scalar2=None,
                                op0=A.is_equal)
                mR = sp.tile([P, NT], F32, tag="mR")
                v.tensor_scalar(out=mR[:], in0=ic[:], scalar1=n - 2.0,
                                scalar2=None, op0=A.is_equal)
                # window start o = clip(i-1, 0, n-4)
                O = pp.tile([P, NT], F32, tag=f"O{dim}")
                v.tensor_scalar(out=O[:], in0=ic[:], scalar1=1.0, scalar2=0.0,
                                op0=A.subtract, op1=A.max)
                v.tensor_scalar(out=O[:], in0=O[:], scalar1=n - 4.0,
                                scalar2=None, op0=A.min)
                O_tiles.append(O)

                w0, w1, w2, w3 = _cubic_weights(nc, sp, f, NT)
                # boundary delta vectors:
                #   left  (i==0):   wL = (w1+2w0, w2-w0, w3, 0)
                #   right (i==n-2): wR = (0, w0, w1-w3, w2+2w3)
                # w' = w + mL*(wL-w) + mR*(wR-w)
                WT = pp.tile([P, NT, 4], F32, tag=f"W{dim}")
                dl = sp.tile([P, NT], F32, tag="dl")
                dr = sp.tile([P, NT], F32, tag="dr")
                acc = sp.tile([P, NT], F32, tag="wacc")

                # component 0: dL0 = w0+w1, dR0 = -w0
                v.tensor_tensor(out=dl[:], in0=w0[:], in1=w1[:], op=A.add)
                v.tensor_tensor(out=dl[:], in0=dl[:], in1=mL[:], op=A.mult)
                v.tensor_tensor(out=dr[:], in0=w0[:], in1=mR[:], op=A.mult)
                v.tensor_tensor(out=acc[:], in0=w0[:], in1=dl[:], op=A.add)
                v.tensor_tensor(out=WT[:, :, 0], in0=acc[:], in1=dr[:],
                                op=A.subtract)
                # component 1: dL1 = w2-w0-w1, dR1 = w0-w1
                v.tensor_tensor(out=dl[:], in0=w2[:], in1=w0[:], op=A.subtract)
                v.tensor_tensor(out=dl[:], in0=dl[:], in1=w1[:], op=A.subtract)
                v.tensor_tensor(out=dl[:], in0=dl[:], in1=mL[:], op=A.mult)
                v.tensor_tensor(out=dr[:], in0=w0[:], in1=w1[:], op=A.subtract)
                v.tensor_tensor(out=dr[:], in0=dr[:], in1=mR[:], op=A.mult)
                v.tensor_tensor(out=acc[:], in0=w1[:], in1=dl[:], op=A.add)
                v.tensor_tensor(out=WT[:, :, 1], in0=acc[:], in1=dr[:], op=A.add)
                # component 2: dL2 = w3-w2, dR2 = w1-w2-w3
                v.tensor_tensor(out=dl[:], in0=w3[:], in1=w2[:], op=A.subtract)
                v.tensor_tensor(out=dl[:], in0=dl[:], in1=mL[:], op=A.mult)
                v.tensor_tensor(out=dr[:], in0=w1[:], in1=w2[:], op=A.subtract)
                v.tensor_tensor(out=dr[:], in0=dr[:], in1=w3[:], op=A.subtract)
                v.tensor_tensor(out=dr[:], in0=dr[:], in1=mR[:], op=A.mult)
                v.tensor_tensor(out=acc[:], in0=w2[:], in1=dl[:], op=A.add)
                v.tensor_tensor(out=WT[:, :, 2], in0=acc[:], in1=dr[:], op=A.add)
                # component 3: dL3 = -w3, dR3 = w2+w3
                v.tensor_tensor(out=dl[:], in0=w3[:], in1=mL[:], op=A.mult)
                v.tensor_tensor(out=dr[:], in0=w2[:], in1=w3[:], op=A.add)
                v.tensor_tensor(out=dr[:], in0=dr[:], in1=mR[:], op=A.mult)
                v.tensor_tensor(out=acc[:], in0=w3[:], in1=dl[:], op=A.subtract)
                v.tensor_tensor(out=WT[:, :, 3], in0=acc[:], in1=dr[:], op=A.add)
                Wd_tiles.append(WT)

            # ---- w-unfolded gather table: unit (t,d,h,wc) = 4 w-points x
            # 32 ch = 128 contiguous f32 (512B).  26624 units fits int16.
            for wc in range(WCELLS):
                nc.sync.dma_start(
                    out=utab[:].rearrange("(g w) e -> g w e", w=WCELLS)[:, wc, :],
                    in_=g_in[:].rearrange("(g x) c -> g (x c)", x=SIZES[3])[
                        :, wc * C : wc * C + UNIT],
                )

            # unit base index = ((ot*16+od)*16+oh)*WCELLS+ow, as int16
            base_f = pp.tile([P, NT], F32)
            v.scalar_tensor_tensor(out=base_f[:], in0=O_tiles[0][:], scalar=16.0,
                                   in1=O_tiles[1][:], op0=A.mult, op1=A.add)
            v.scalar_tensor_tensor(out=base_f[:], in0=base_f[:], scalar=16.0,
                                   in1=O_tiles[2][:], op0=A.mult, op1=A.add)
            v.scalar_tensor_tensor(out=base_f[:], in0=base_f[:],
                                   scalar=float(WCELLS), in1=O_tiles[3][:],
                                   op0=A.mult, op1=A.add)
            base_i = pp.tile([P, NT], I16)
            v.tensor_copy(out=base_i[:], in_=base_f[:])
            # bounce to DRAM in query order so per-tile loads can re-wrap it
            # into dma_gather's 16-partition index layout
            nc.sync.dma_start(
                out=bscr[:].rearrange("(t p) -> p t", p=P), in_=base_i[:])

            # window offsets i*(16*16*WCELLS) + j*(16*WCELLS) + k*WCELLS,
            # (i,j,k) C-order, replicated on all partitions
            offs = pp.tile([P, 64], I16)
            nc.gpsimd.iota(
                out=offs[:],
                pattern=[[256 * WCELLS, 4], [16 * WCELLS, 4], [WCELLS, 4]],
                base=0, channel_multiplier=0)

            # re-load bases wrapped for dma_gather's index layout:
            # bwall[p', t, jj] = base[query t*128 + jj*16 + p'%16], i.e. each
            # 16-partition Q7 group holds a replica (8 replication DMAs).
            bwall = pp.tile([P, NT, 8], I16)
            for g2 in range(8):
                nc.sync.dma_start(
                    out=bwall[g2 * 16 : (g2 + 1) * 16, :, :],
                    in_=bass.AP(bscr, 0, [[1, 16], [128, NT], [16, 8]]),
                )

            # ---------------- Phase B: per-tile gather + reduce ------------
            # ACT (ScalarE) can only do out = in*scale (per-partition scalar),
            # so it takes pure products; DVE (VectorE) does the FMA/add chain.
            wt, wd, wh, ww = Wd_tiles
            Copy = mybir.ActivationFunctionType.Copy
            for t in range(NT):
                # wrapped index layout: idx[p, w*8+jj] = base[q=jj*16+p%16]
                # + offs[w]; dma_gather reads list pos n at [n%16, n//16] and
                # writes gather n to partition n%128, slot n//128.
                idx = sp.tile([P, 64, 8], I16, tag="idx")
                v.tensor_tensor(
                    out=idx[:],
                    in0=bwall[:, t : t + 1, :].to_broadcast([P, 64, 8]),
                    in1=offs[:].rearrange("p (w o) -> p w o", o=1).to_broadcast(
                        [P, 64, 8]),
                    op=A.add,
                )
                g = gp.tile([P, 64, UNIT], F32, tag="g")
                # the SWDGE descriptor ring holds 1024 descriptors, so split
                # the tile's 8192-window gather into 8 sub-gathers
                for k in (range(0) if no_gather else range(8)):
                    nc.gpsimd.dma_gather(
                        out_ap=g[:, 8 * k : 8 * (k + 1), :],
                        in_ap=utab[:],
                        idxs_ap=idx[:, 8 * k : 8 * (k + 1), :].rearrange(
                            "p w j -> p (w j)"),
                        num_idxs=P * 8,
                        num_idxs_reg=P * 8,
                        elem_size=UNIT,
                    )

                if no_reduce:
                    otile0 = rp.tile([P, C], F32, tag="otile")
                    v.tensor_copy(out=otile0[:], in_=g[:, 0, 0:C])
                    nc.sync.dma_start(out=out[t * P : (t + 1) * P, :],
                                      in_=otile0[:])
                    continue

                def xsl(l):  # gathered l-slice [P, 64, C]
                    return g[:, :, l * C : (l + 1) * C]

                # ACT computes all weighted products (it owns the strided
                # reads); DVE does only contiguous adds.  Per-stage product
                # tiles live in the prod pool (bufs=3) for overlap.
                def stage(src_fn, wtile, nwin, ptag, stag):
                    prods = []
                    for l in range(4):
                        pl = prp.tile([P, nwin, C], F32, tag=ptag)
                        nc.scalar.activation(out=pl[:], in_=src_fn(l),
                                             func=Copy,
                                             scale=wtile[:, t, l : l + 1])
                        prods.append(pl)
                    s0 = rp.tile([P, nwin, C], F32, tag=stag + "0")
                    s1 = rp.tile([P, nwin, C], F32, tag=stag + "1")
                    v.tensor_tensor(out=s0[:], in0=prods[0][:],
                                    in1=prods[1][:], op=A.add)
                    v.tensor_tensor(out=s1[:], in0=s0[:], in1=prods[2][:],
                                    op=A.add)
                    s2 = rp.tile([P, nwin, C], F32, tag=stag + "0")
                    v.tensor_tensor(out=s2[:], in0=s1[:], in1=prods[3][:],
                                    op=A.add)
                    return s2

                # stage w (l): y[win, c] = sum_l g[win, l, c]*ww_l
                y = stage(xsl, ww, 64, "pw", "yw")
                yv = y[:].rearrange("p (ij k) c -> p ij k c", k=4)
                # stage h (k): z[ij, c] = sum_k y[ij, k, c]*wh_k
                z = stage(lambda k: yv[:, :, k, :], wh, 16, "ph", "zh")
                zv = z[:].rearrange("p (i j) c -> p i j c", j=4)
                # stage d (j): small -> DVE FMA chain
                d0 = rp.tile([P, 4, C], F32, tag="dd0")
                d1 = rp.tile([P, 4, C], F32, tag="dd1")
                v.tensor_scalar(out=d0[:], in0=zv[:, :, 0, :],
                                scalar1=wd[:, t, 0:1], scalar2=None, op0=A.mult)
                v.scalar_tensor_tensor(out=d1[:], in0=zv[:, :, 1, :],
                                       scalar=wd[:, t, 1:2], in1=d0[:],
                                       op0=A.mult, op1=A.add)
                v.scalar_tensor_tensor(out=d0[:], in0=zv[:, :, 2, :],
                                       scalar=wd[:, t, 2:3], in1=d1[:],
                                       op0=A.mult, op1=A.add)
                v.scalar_tensor_tensor(out=d1[:], in0=zv[:, :, 3, :],
                                       scalar=wd[:, t, 3:4], in1=d0[:],
                                       op0=A.mult, op1=A.add)
                dv = d1[:]
                # stage t (i): small -> DVE FMA chain
                o0 = rp.tile([P, C], F32, tag="oo0")
                o1 = rp.tile([P, C], F32, tag="oo1")
                v.tensor_scalar(out=o0[:], in0=dv[:, 0, :],
                                scalar1=wt[:, t, 0:1], scalar2=None, op0=A.mult)
                v.scalar_tensor_tensor(out=o1[:], in0=dv[:, 1, :],
                                       scalar=wt[:, t, 1:2], in1=o0[:],
                                       op0=A.mult, op1=A.add)
                v.scalar_tensor_tensor(out=o0[:], in0=dv[:, 2, :],
                                       scalar=wt[:, t, 2:3], in1=o1[:],
                                       op0=A.mult, op1=A.add)
                o2 = rp.tile([P, C], F32, tag="oo2")
                v.scalar_tensor_tensor(out=o2[:], in0=dv[:, 3, :],
                                       scalar=wt[:, t, 3:4], in1=o0[:],
                                       op0=A.mult, op1=A.add)
                nc.sync.dma_start(out=out[t * P : (t + 1) * P, :], in_=o2[:])

    nc.compile()
    return nc


def _get_nc():
    global _CACHED_NC
    if _CACHED_NC is None:
        _CACHED_NC = _build_nc()
    return _CACHED_NC


def kernel(u: np.ndarray, grid: np.ndarray) -> np.ndarray:
    u = np.ascontiguousarray(np.asarray(u, dtype=np.float32))
    grid = np.ascontiguousarray(np.asarray(grid, dtype=np.float32))
    gflat = grid.reshape(-1, C)
    nc = _get_nc()
    in_maps = [
        {"u": u[c * BSH : (c + 1) * BSH], "grid": gflat}
        for c in range(NCORES)
    ]
    res = run_bass_kernel_spmd(nc, in_maps, list(range(NCORES)))
    return np.concatenate([res.results[c]["out"] for c in range(NCORES)], axis=0)


if __name__ == "__main__":
    rng = np.random.default_rng(0)
    u = rng.random((B, 4), dtype=np.float32)
    grid = rng.standard_normal((*SIZES, C), dtype=np.float32)
    out = kernel(u, grid)
    print(out.shape, out.dtype)

